# revision 40
# baseline (speedup 1.0000x reference)
"""Trainium2 Bass kernel for a BasicTransformerBlock (self-attn + cross-attn +
GEGLU FFN), sharded over 8 NeuronCores.

Sharding: data-parallel over batch (2) x sequence chunks (4): core c handles
batch c//4, query rows [(c%4)*1024, (c%4+1)*1024). Each core recomputes
LN1+K/V over its batch's full 4096-row sequence (needed for self-attention)
and produces its own 1024-row output chunk. No collectives.

Layout convention on device: residual stream is token-major f32 [128_tok, 512]
tiles; matmul operands are bf16; activations are transposed to feature-major
[feat_part, tok_free] with PE transposes so the tensor engine can contract
over features; attention probabilities stay feature-major [key_part, query]
so exp'd tiles feed attn@v directly as the stationary operand.
"""

import numpy as np
import ml_dtypes
from contextlib import ExitStack

import concourse.bass as bass
import concourse.tile as tile
from concourse import bacc, mybir
from concourse import bass_utils

F32 = mybir.dt.float32
BF16 = mybir.dt.bfloat16
AF = mybir.ActivationFunctionType
ALU = mybir.AluOpType

# problem constants (hardcoded per the harness contract)
B = 2
N = 4096          # self-attn sequence length (per batch)
NC = 1024         # per-core query chunk
D = 512           # model dim
H = 8             # heads
DH = 64           # head dim
M = 256           # context length
CD = 768          # context dim
FF = 2048         # GEGLU inner dim
LN_EPS = 1e-5
SCALE = DH ** -0.5

NT = N // 128      # 32 x_full tiles
NCT = NC // 128    # 8 own tiles


def _lnt(tc, ctx, nc, src_tiles, hT, ident, psum_tr, stat_pool, h_pool,
         name=""):
    """LayerNorm (token-major f32 src tiles [128,512]) -> bf16, then PE
    transpose into feature-major hT tiles: hT[cc][:, t*128:(t+1)*128].
    src_tiles: list of SBUF tiles [128, 512] f32."""
    nt = len(src_tiles)
    hs = []
    for t, xt in enumerate(src_tiles):
        stats = stat_pool.tile([128, 6], F32, name=f"st{name}", tag="st")
        nc.vector.bn_stats(stats[:], xt[:])
        aggr = stat_pool.tile([128, 2], F32, name=f"ag{name}", tag="ag")
        nc.vector.bn_aggr(aggr[:], stats[:])
        veps = stat_pool.tile([128, 1], F32, name=f"ve{name}", tag="ve")
        nc.vector.tensor_scalar_add(veps[:], aggr[:, 1:2], LN_EPS)
        rvar = stat_pool.tile([128, 1], F32, name=f"rv{name}", tag="rv")
        nc.vector.reciprocal(rvar[:], veps[:])
        rstd = stat_pool.tile([128, 1], F32, name=f"rs{name}", tag="rs")
        nc.scalar.sqrt(rstd[:], rvar[:])
        h = h_pool.tile([128, D], BF16, name=f"h{name}", tag="h")
        nc.vector.tensor_scalar(h[:], xt[:], aggr[:, 0:1], rstd[:],
                                op0=ALU.subtract, op1=ALU.mult)
        hs.append(h)
    # transpose groups of 4 token-tiles at a time
    for tg in range((nt + 3) // 4):
        grp = list(range(tg * 4, min(tg * 4 + 4, nt)))
        for cc in range(4):
            ps = psum_tr.tile([128, 512], BF16, name=f"pst{name}", tag="pst")
            for k, t in enumerate(grp):
                nc.tensor.transpose(ps[:, k * 128:(k + 1) * 128],
                                    hs[t][:, cc * 128:(cc + 1) * 128],
                                    ident[:])
            w = len(grp) * 128
            nc.scalar.copy(hT[cc][:, tg * 512:tg * 512 + w], ps[:, :w])


def build_nc(debug_taps=False):
    nc = bacc.Bacc("TRN2", target_bir_lowering=False, debug=False,
                   enable_asserts=False, num_devices=8)
    dbg = {}

    def tap(name, tiles, rows=128):
        """DMA a list of SBUF tiles out to a debug DRAM tensor."""
        if not debug_taps:
            return
        ap0 = tiles[0][:]
        cols = ap0.shape[-1]
        dt = ap0.dtype
        t = nc.dram_tensor(f"dbg_{name}", [len(tiles) * rows, cols], dt,
                           kind="ExternalOutput").ap()
        for i, ti in enumerate(tiles):
            nc.sync.dma_start(t[i * rows:(i + 1) * rows, :], ti[:rows, :])
        dbg[name] = t
    dt_in = {}

    def din(name, shape, dt):
        dt_in[name] = nc.dram_tensor(name, shape, dt, kind="ExternalInput").ap()
        return dt_in[name]

    x_full = din("x_full", [N, D], F32)
    x_own = din("x_own", [NC, D], F32)
    ctxT = din("ctxT", [CD, M], BF16)
    wq1 = din("wq1", [D, D], BF16)
    wk1 = din("wk1", [D, D], BF16)
    wv1 = din("wv1", [D, D], BF16)
    wo1 = din("wo1", [D, D], BF16)
    wq2 = din("wq2", [D, D], BF16)
    wk2 = din("wk2", [CD, D], BF16)
    wv2 = din("wv2", [CD, D], BF16)
    wo2 = din("wo2", [D, D], BF16)
    wfi = din("wfi", [D, 2 * FF], BF16)
    wfo = din("wfo", [FF, D], BF16)
    ident_d = din("ident", [128, 128], BF16)
    y = nc.dram_tensor("y", [NC, D], F32, kind="ExternalOutput").ap()

    xf_t = x_full.rearrange("(t p) d -> t p d", p=128)
    xo_t = x_own.rearrange("(t p) d -> t p d", p=128)

    with tile.TileContext(nc) as tc, ExitStack() as top:
        const = top.enter_context(tc.tile_pool(name="const", bufs=1))
        ident = const.tile([128, 128], BF16)
        nc.sync.dma_start(ident[:], ident_d[:])

        y1_pool = top.enter_context(tc.tile_pool(name="y1", bufs=1))
        y1 = [y1_pool.tile([128, D], F32, name=f"y1_{i}", tag=f"y1_{i}")
              for i in range(NCT)]
        wq2p = top.enter_context(tc.tile_pool(name="wq2p", bufs=1))
        wq2_t = [wq2p.tile([128, D], BF16, name=f"wq2_{i}", tag=f"wq2_{i}")
                 for i in range(4)]
        for i in range(4):
            nc.sync.dma_start(wq2_t[i][:], wq2[i * 128:(i + 1) * 128, :])
        h2T_pool = top.enter_context(tc.tile_pool(name="h2T", bufs=1))
        h2T = [h2T_pool.tile([128, NC], BF16, name=f"h2T{i}", tag=f"h2T{i}")
               for i in range(4)]
        q2T_pool = top.enter_context(tc.tile_pool(name="q2T", bufs=1))
        q2T = [q2T_pool.tile([128, NC], BF16, name=f"q2T{i}", tag=f"q2T{i}")
               for i in range(4)]

        # ---------------- Phase A: self-attention ----------------
        with ExitStack() as pa:
            w1 = pa.enter_context(tc.tile_pool(name="w1", bufs=1))
            wq1_t = [w1.tile([128, D], BF16, name=f"wq1_{i}", tag=f"wq1_{i}") for i in range(4)]
            wk1_t = [w1.tile([128, D], BF16, name=f"wk1_{i}", tag=f"wk1_{i}") for i in range(4)]
            wv1_t = [w1.tile([128, D], BF16, name=f"wv1_{i}", tag=f"wv1_{i}") for i in range(4)]
            wo1_t = [w1.tile([128, D], BF16, name=f"wo1_{i}", tag=f"wo1_{i}") for i in range(4)]
            kT_pool = pa.enter_context(tc.tile_pool(name="kT", bufs=1))
            kT = [kT_pool.tile([128, N], BF16, name=f"kT{i}", tag=f"kT{i}")
                  for i in range(4)]
            va_pool = pa.enter_context(tc.tile_pool(name="va", bufs=1))
            va = [va_pool.tile([128, H * (DH + 1)], BF16, name=f"va{i}",
                               tag=f"va{i}") for i in range(NT)]
            qT_pool = pa.enter_context(tc.tile_pool(name="qT", bufs=1))
            qT = [qT_pool.tile([128, NC], BF16, name=f"qT{i}", tag=f"qT{i}")
                  for i in range(4)]
            xo_pool = pa.enter_context(tc.tile_pool(name="xo", bufs=1))
            xo = [xo_pool.tile([128, D], F32, name=f"xo{i}", tag=f"xo{i}")
                  for i in range(NCT)]
            for i in range(NCT):
                nc.sync.dma_start(xo[i][:], xo_t[i])
            oT_pool = pa.enter_context(tc.tile_pool(name="oT", bufs=1))
            oT = [[oT_pool.tile([128, 512], BF16, name=f"oT{d}_{i}",
                                tag=f"oT{d}_{i}") for i in range(2)]
                  for d in range(4)]

            # --- projections (scoped so h1T + psum free before attention) ---
            with ExitStack() as pp:
                h1T_pool = pp.enter_context(tc.tile_pool(name="h1T", bufs=1))
                h1T = [h1T_pool.tile([128, N], BF16, name=f"h1T{i}",
                                     tag=f"h1T{i}") for i in range(4)]
                hoT_pool = pp.enter_context(tc.tile_pool(name="hoT", bufs=1))
                hoT = [hoT_pool.tile([128, NC], BF16, name=f"hoT{i}",
                                     tag=f"hoT{i}") for i in range(4)]
                psum_tr = pp.enter_context(
                    tc.tile_pool(name="ptr", bufs=2, space="PSUM"))
                stat_pool = pp.enter_context(tc.tile_pool(name="stat", bufs=4))
                h_pool = pp.enter_context(tc.tile_pool(name="hp", bufs=6))
                xf_pool = pp.enter_context(tc.tile_pool(name="xf", bufs=4))

                xf_tiles = []
                for t in range(NT):
                    xt = xf_pool.tile([128, D], F32, name=f"xf{t}", tag="xf")
                    nc.sync.dma_start(xt[:], xf_t[t])
                    xf_tiles.append(xt)
                # weight DMAs issued after x so they don't delay the LN path
                for i in range(4):
                    nc.sync.dma_start(wq1_t[i][:], wq1[i * 128:(i + 1) * 128, :])
                    nc.sync.dma_start(wk1_t[i][:], wk1[i * 128:(i + 1) * 128, :])
                    nc.sync.dma_start(wv1_t[i][:], wv1[i * 128:(i + 1) * 128, :])
                    nc.sync.dma_start(wo1_t[i][:], wo1[i * 128:(i + 1) * 128, :])
                psum_pj = pp.enter_context(
                    tc.tile_pool(name="ppj", bufs=4, space="PSUM"))
                # fused LN1 + transpose + K/V projection per 512-token group:
                # keeps the PE fed with projection matmuls while the DVE works
                # on the next group's layernorm
                for tg in range(8):
                    grp = xf_tiles[tg * 4:(tg + 1) * 4]
                    hs = []
                    for xt in grp:
                        stats = stat_pool.tile([128, 6], F32, name="st1",
                                               tag="st")
                        nc.vector.bn_stats(stats[:], xt[:])
                        aggr = stat_pool.tile([128, 2], F32, name="ag1",
                                              tag="ag")
                        nc.vector.bn_aggr(aggr[:], stats[:])
                        veps = stat_pool.tile([128, 1], F32, name="ve1",
                                              tag="ve")
                        nc.vector.tensor_scalar_add(veps[:], aggr[:, 1:2],
                                                    LN_EPS)
                        rvar = stat_pool.tile([128, 1], F32, name="rv1",
                                              tag="rv")
                        nc.vector.reciprocal(rvar[:], veps[:])
                        rstd = stat_pool.tile([128, 1], F32, name="rs1",
                                              tag="rs")
                        nc.scalar.sqrt(rstd[:], rvar[:])
                        h = h_pool.tile([128, D], BF16, name="h1", tag="h")
                        nc.vector.tensor_scalar(h[:], xt[:], aggr[:, 0:1],
                                                rstd[:], op0=ALU.subtract,
                                                op1=ALU.mult)
                        hs.append(h)
                    for cc in range(4):
                        ps = psum_tr.tile([128, 512], BF16, name="pstr",
                                          tag="pst")
                        for k in range(4):
                            nc.tensor.transpose(
                                ps[:, k * 128:(k + 1) * 128],
                                hs[k][:, cc * 128:(cc + 1) * 128], ident[:])
                        nc.scalar.copy(
                            h1T[cc][:, tg * 512:(tg + 1) * 512], ps[:])
                    for dc in range(4):
                        ps = psum_pj.tile([128, 512], F32, name="pk", tag="pj")
                        for cc in range(4):
                            nc.tensor.matmul(
                                ps[:],
                                lhsT=wk1_t[cc][:, dc * 128:(dc + 1) * 128],
                                rhs=h1T[cc][:, tg * 512:(tg + 1) * 512],
                                start=(cc == 0), stop=(cc == 3))
                        nc.scalar.copy(
                            kT[dc][:, tg * 512:(tg + 1) * 512], ps[:])
                    for jt in range(tg * 4, (tg + 1) * 4):
                        ps = psum_pj.tile([128, 512], F32, name="pv", tag="pj")
                        for cc in range(4):
                            nc.tensor.matmul(
                                ps[:],
                                lhsT=h1T[cc][:, jt * 128:(jt + 1) * 128],
                                rhs=wv1_t[cc][:],
                                start=(cc == 0), stop=(cc == 3))
                        va_r = va[jt].rearrange("p (h e) -> p h e", e=DH + 1)
                        nc.vector.tensor_copy(
                            va_r[:, :, 0:DH],
                            ps[:].rearrange("p (h e) -> p h e", e=DH))
                        nc.gpsimd.memset(va_r[:, :, DH:DH + 1], 1.0)
                _lnt(tc, pp, nc, xo, hoT, ident, psum_tr, stat_pool,
                     h_pool, name="lo")
                # q1T
                for jg in range(NC // 512):
                    for dc in range(4):
                        ps = psum_pj.tile([128, 512], F32, name="pq", tag="pj")
                        for cc in range(4):
                            nc.tensor.matmul(
                                ps[:],
                                lhsT=wq1_t[cc][:, dc * 128:(dc + 1) * 128],
                                rhs=hoT[cc][:, jg * 512:(jg + 1) * 512],
                                start=(cc == 0), stop=(cc == 3))
                        nc.scalar.copy(
                            qT[dc][:, jg * 512:(jg + 1) * 512], ps[:])
                tap("h1T", h1T)
                tap("hoT", hoT)
            # --- attention main loop: ic-major with paired heads.
            # PSUM pools are scoped per ic chunk so that finish_ic (to_out1,
            # LN2, h2T, q2T) gets banks and overlaps the next chunk's
            # attention via Tile's dependency tracking. ---
            o_pool = pa.enter_context(tc.tile_pool(name="o_t", bufs=1))
            o_t = [[o_pool.tile([128, 4 * DH], BF16, name=f"o{h}_{i}",
                                tag=f"o{h}_{i}") for i in range(2)]
                   for h in range(H)]
            sm_pool = pa.enter_context(tc.tile_pool(name="sm", bufs=4))
            st2 = pa.enter_context(tc.tile_pool(name="st2", bufs=4))
            h2_pool = pa.enter_context(tc.tile_pool(name="hp2", bufs=6))
            pP = pa.enter_context(tc.tile_pool(name="pP", bufs=13))
            groups = [list(range(g * 3, min(g * 3 + 3, NT)))
                      for g in range((NT + 2) // 3)]
            WIN = 2

            def attn1_ic(ic, psS, psA):
                for hp in range(4):
                    oacc = [psA.tile([65, 512], F32, name=f"oa{hh}",
                                     tag="oa") for hh in range(2)]
                    for w0 in range(0, len(groups), WIN):
                        win = groups[w0:w0 + WIN]
                        pw = []
                        for grp in win:
                            w = len(grp) * 512
                            for hh in range(2):
                                base = 64 * hh
                                ps = psS.tile([128, 1536], F32, name="sim",
                                              tag="sim")
                                for k, jc in enumerate(grp):
                                    nc.tensor.matmul(
                                        ps[:, k * 512:(k + 1) * 512],
                                        lhsT=kT[hp][base:base + 64,
                                                    jc * 128:(jc + 1) * 128],
                                        rhs=qT[hp][base:base + 64,
                                                   ic * 512:(ic + 1) * 512],
                                        start=True, stop=True)
                                p = pP.tile([128, 1536], BF16, name="p",
                                            tag="p")
                                nc.scalar.activation(p[:, :w], ps[:, :w],
                                                     AF.Exp)
                                pw.append((hh, grp, p))
                        for hh, grp, p in pw:
                            h = 2 * hp + hh
                            for k, jc in enumerate(grp):
                                nc.tensor.matmul(
                                    oacc[hh][:],
                                    lhsT=va[jc][:, h * 65:h * 65 + 65],
                                    rhs=p[:, k * 512:(k + 1) * 512],
                                    start=(jc == 0), stop=(jc == NT - 1),
                                    skip_group_check=True)
                    for hh in range(2):
                        h = 2 * hp + hh
                        oc = sm_pool.tile([65, 512], BF16, name="oc",
                                          tag="oc")
                        nc.vector.tensor_copy(oc[:], oacc[hh][:])
                        pst = psA.tile([128, 264], BF16, name="pstt",
                                       tag="oa")
                        for m in range(4):
                            nc.tensor.transpose(
                                pst[:, m * 66:m * 66 + 65],
                                oc[:, m * 128:(m + 1) * 128],
                                ident[0:65, 0:65])
                        recip = sm_pool.tile([128, 4], F32, name="rc",
                                             tag="rc")
                        nc.vector.reciprocal(
                            recip[:],
                            pst[:, 0:264].rearrange(
                                "p (k e) -> p k e", e=66)[:, :, 64:65])
                        for m in range(4):
                            nc.vector.tensor_scalar(
                                o_t[h][ic][:, m * 64:(m + 1) * 64],
                                pst[:, m * 66:m * 66 + 64],
                                recip[:, m:m + 1], None, op0=ALU.mult)

            def finish_ic(ic, pfin):
                psU = pfin.enter_context(
                    tc.tile_pool(name=f"psU{ic}", bufs=2, space="PSUM"))
                for h in range(H):
                    dc, base = h // 2, 64 * (h % 2)
                    ps = psU.tile([64, 512], BF16, name="psO", tag="u")
                    for m in range(4):
                        nc.tensor.transpose(
                            ps[:, m * 128:(m + 1) * 128],
                            o_t[h][ic][:, m * 64:(m + 1) * 64], ident[:])
                    nc.vector.tensor_copy(
                        oT[dc][ic][base:base + 64, :], ps[:])
                h2s = []
                for m in range(4):
                    it = ic * 4 + m
                    ps = psU.tile([128, 512], F32, name="pu", tag="u")
                    for dc in range(4):
                        nc.tensor.matmul(
                            ps[:],
                            lhsT=oT[dc][ic][:, m * 128:(m + 1) * 128],
                            rhs=wo1_t[dc][:],
                            start=(dc == 0), stop=(dc == 3))
                    nc.vector.tensor_add(y1[it][:], ps[:], xo[it][:])
                    stats = st2.tile([128, 6], F32, name="st2", tag="st")
                    nc.vector.bn_stats(stats[:], y1[it][:])
                    aggr = st2.tile([128, 2], F32, name="ag2", tag="ag")
                    nc.vector.bn_aggr(aggr[:], stats[:])
                    veps = st2.tile([128, 1], F32, name="ve2", tag="ve")
                    nc.vector.tensor_scalar_add(veps[:], aggr[:, 1:2],
                                                LN_EPS)
                    rvar = st2.tile([128, 1], F32, name="rv2", tag="rv")
                    nc.vector.reciprocal(rvar[:], veps[:])
                    rstd = st2.tile([128, 1], F32, name="rs2", tag="rs")
                    nc.scalar.sqrt(rstd[:], rvar[:])
                    h2 = h2_pool.tile([128, D], BF16, name="h2", tag="h")
                    nc.vector.tensor_scalar(h2[:], y1[it][:],
                                            aggr[:, 0:1], rstd[:],
                                            op0=ALU.subtract, op1=ALU.mult)
                    h2s.append(h2)
                for cc in range(4):
                    ps = psU.tile([128, 512], BF16, name="ph2", tag="u")
                    for k in range(4):
                        nc.tensor.transpose(
                            ps[:, k * 128:(k + 1) * 128],
                            h2s[k][:, cc * 128:(cc + 1) * 128], ident[:])
                    nc.scalar.copy(
                        h2T[cc][:, ic * 512:(ic + 1) * 512], ps[:])
                for dc in range(4):
                    ps = psU.tile([128, 512], F32, name="pq2", tag="u")
                    for cc in range(4):
                        nc.tensor.matmul(
                            ps[:],
                            lhsT=wq2_t[cc][:, dc * 128:(dc + 1) * 128],
                            rhs=h2T[cc][:, ic * 512:(ic + 1) * 512],
                            start=(cc == 0), stop=(cc == 3))
                    nc.scalar.copy(
                        q2T[dc][:, ic * 512:(ic + 1) * 512], ps[:])

            with ExitStack() as pat:
                psS = pat.enter_context(
                    tc.tile_pool(name="psS", bufs=2, space="PSUM"))
                psA = pat.enter_context(
                    tc.tile_pool(name="psA", bufs=2, space="PSUM"))
                attn1_ic(0, psS, psA)
                attn1_ic(1, psS, psA)
            with ExitStack() as pfin:
                finish_ic(0, pfin)
                finish_ic(1, pfin)

        tap("y1", y1)

        # ---------------- Phase B: cross-attention + FFN ----------------
        with ExitStack() as pb:
            wf = pb.enter_context(tc.tile_pool(name="wf", bufs=1))
            wfi_t = [wf.tile([128, 2 * FF], BF16, name=f"wfi{i}", tag=f"wfi{i}")
                     for i in range(4)]
            wfo_t = [wf.tile([128, D], BF16, name=f"wfo{i}", tag=f"wfo{i}")
                     for i in range(FF // 128)]
            for i in range(4):
                nc.sync.dma_start(wfi_t[i][:], wfi[i * 128:(i + 1) * 128, :])
            for i in range(FF // 128):
                nc.sync.dma_start(wfo_t[i][:], wfo[i * 128:(i + 1) * 128, :])

            w2 = pb.enter_context(tc.tile_pool(name="w2", bufs=1))
            wk2_t = [w2.tile([128, D], BF16, name=f"wk2_{i}", tag=f"wk2_{i}") for i in range(6)]
            wv2_t = [w2.tile([128, D], BF16, name=f"wv2_{i}", tag=f"wv2_{i}") for i in range(6)]
            wo2_t = [w2.tile([128, D], BF16, name=f"wo2_{i}", tag=f"wo2_{i}") for i in range(4)]
            ctx_t = [w2.tile([128, M], BF16, name=f"ctx{i}", tag=f"ctx{i}") for i in range(6)]
            for i in range(4):
                nc.sync.dma_start(wo2_t[i][:], wo2[i * 128:(i + 1) * 128, :])
            for i in range(6):
                nc.sync.dma_start(wk2_t[i][:], wk2[i * 128:(i + 1) * 128, :])
                nc.sync.dma_start(wv2_t[i][:], wv2[i * 128:(i + 1) * 128, :])
                nc.sync.dma_start(ctx_t[i][:], ctxT[i * 128:(i + 1) * 128, :])

            y2_pool = pb.enter_context(tc.tile_pool(name="y2", bufs=1))
            y2 = [y2_pool.tile([128, D], F32, name=f"y2_{i}", tag=f"y2_{i}")
                  for i in range(NCT)]

            k2T_pool = pb.enter_context(tc.tile_pool(name="k2T", bufs=1))
            k2T = [k2T_pool.tile([128, M], BF16, name=f"k2T{i}", tag=f"k2T{i}")
                   for i in range(4)]
            va2_pool = pb.enter_context(tc.tile_pool(name="va2", bufs=1))
            va2 = [va2_pool.tile([128, H * (DH + 1)], BF16, name=f"va2_{i}",
                                 tag=f"va2_{i}") for i in range(2)]
            o2T_pool = pb.enter_context(tc.tile_pool(name="o2T", bufs=1))
            o2T = [[o2T_pool.tile([128, 512], BF16, name=f"o2T{d}_{i}",
                                  tag=f"o2T{d}_{i}") for i in range(2)]
                   for d in range(4)]

            # --- projections for cross-attn (k2/v2 from context only;
            # h2T/q2T were produced during the attention overlap) ---
            with ExitStack() as pp2:
                psum_p2 = pp2.enter_context(
                    tc.tile_pool(name="pp2", bufs=4, space="PSUM"))
                for dc in range(4):
                    ps = psum_p2.tile([128, M], F32, name="pk2", tag="p2")
                    for cc in range(6):
                        nc.tensor.matmul(
                            ps[:],
                            lhsT=wk2_t[cc][:, dc * 128:(dc + 1) * 128],
                            rhs=ctx_t[cc][:],
                            start=(cc == 0), stop=(cc == 5))
                    nc.vector.tensor_copy(k2T[dc][:], ps[:])
                for jt in range(2):
                    ps = psum_p2.tile([128, 512], F32, name="pv2", tag="p2")
                    for cc in range(6):
                        nc.tensor.matmul(
                            ps[:],
                            lhsT=ctx_t[cc][:, jt * 128:(jt + 1) * 128],
                            rhs=wv2_t[cc][:],
                            start=(cc == 0), stop=(cc == 5))
                    va_r = va2[jt].rearrange("p (h e) -> p h e", e=DH + 1)
                    nc.vector.tensor_copy(
                        va_r[:, :, 0:DH],
                        ps[:].rearrange("p (h e) -> p h e", e=DH))
                    nc.gpsimd.memset(va_r[:, :, DH:DH + 1], 1.0)

            # --- cross-attention loop (2 key tiles) ---
            o2_pool = pb.enter_context(tc.tile_pool(name="o2_t", bufs=1))
            o2_t = [[o2_pool.tile([128, 4 * DH], BF16, name=f"o2{h}_{i}",
                                  tag=f"o2{h}_{i}") for i in range(2)]
                    for h in range(H)]
            with ExitStack() as pat2:
                psS2 = pat2.enter_context(
                    tc.tile_pool(name="psS2", bufs=2, space="PSUM"))
                psA2 = pat2.enter_context(
                    tc.tile_pool(name="psA2", bufs=2, space="PSUM"))
                pP2 = pat2.enter_context(tc.tile_pool(name="pP2", bufs=4))
                sm2 = pat2.enter_context(tc.tile_pool(name="sm2", bufs=4))
                for hp in range(4):
                    for ic in range(2):
                        oacc = [psA2.tile([65, 512], F32, name=f"o2a{hh}",
                                          tag="o2a") for hh in range(2)]
                        for hh in range(2):
                            h = 2 * hp + hh
                            base = 64 * hh
                            ps = psS2.tile([128, 1024], F32, name="sim2",
                                           tag="sim2")
                            for jc in range(2):
                                nc.tensor.matmul(
                                    ps[:, jc * 512:(jc + 1) * 512],
                                    lhsT=k2T[hp][base:base + 64,
                                                 jc * 128:(jc + 1) * 128],
                                    rhs=q2T[hp][base:base + 64,
                                                ic * 512:(ic + 1) * 512],
                                    start=True, stop=True)
                            p = pP2.tile([128, 1024], BF16, name="p2", tag="p2")
                            nc.scalar.activation(p[:], ps[:], AF.Exp)
                            for jc in range(2):
                                nc.tensor.matmul(
                                    oacc[hh][:],
                                    lhsT=va2[jc][:, h * 65:h * 65 + 65],
                                    rhs=p[:, jc * 512:(jc + 1) * 512],
                                    start=(jc == 0), stop=(jc == 1),
                                    skip_group_check=True)
                        for hh in range(2):
                            h = 2 * hp + hh
                            oc = sm2.tile([65, 512], BF16, name="oc2",
                                          tag="oc2")
                            nc.vector.tensor_copy(oc[:], oacc[hh][:])
                            pst = psA2.tile([128, 264], BF16, name="pstt2",
                                            tag="o2a")
                            for m in range(4):
                                nc.tensor.transpose(
                                    pst[:, m * 66:m * 66 + 65],
                                    oc[:, m * 128:(m + 1) * 128],
                                    ident[0:65, 0:65])
                            recip = sm2.tile([128, 4], F32, name="rc2",
                                             tag="rc2")
                            nc.vector.reciprocal(
                                recip[:],
                                pst[:, 0:264].rearrange(
                                    "p (k e) -> p k e", e=66)[:, :, 64:65])
                            for m in range(4):
                                nc.vector.tensor_scalar(
                                    o2_t[h][ic][:, m * 64:(m + 1) * 64],
                                    pst[:, m * 66:m * 66 + 64],
                                    recip[:, m:m + 1], None, op0=ALU.mult)

            with ExitStack() as pot2:
                psO2 = pot2.enter_context(
                    tc.tile_pool(name="psO2", bufs=4, space="PSUM"))
                for h in range(H):
                    dc, base = h // 2, 64 * (h % 2)
                    for ic in range(2):
                        ps = psO2.tile([64, 512], BF16, name="pso2", tag="pso2")
                        for m in range(4):
                            nc.tensor.transpose(
                                ps[:, m * 128:(m + 1) * 128],
                                o2_t[h][ic][:, m * 64:(m + 1) * 64],
                                ident[:])
                        nc.vector.tensor_copy(
                            o2T[dc][ic][base:base + 64, :], ps[:])

            with ExitStack() as pto2:
                psU2 = pto2.enter_context(
                    tc.tile_pool(name="psU2", bufs=2, space="PSUM"))
                for ic in range(2):
                    for m in range(4):
                        it = ic * 4 + m
                        ps = psU2.tile([128, 512], F32, name="pu2", tag="pu2")
                        for dc in range(4):
                            nc.tensor.matmul(
                                ps[:],
                                lhsT=o2T[dc][ic][:, m * 128:(m + 1) * 128],
                                rhs=wo2_t[dc][:],
                                start=(dc == 0), stop=(dc == 3))
                        nc.vector.tensor_add(y2[it][:], ps[:], y1[it][:])

            tap("k2T", k2T)
            tap("q2T", q2T)
            tap("va2", va2)
            tap("o2_t", [o2_t[h][i] for h in range(H) for i in range(2)])
            tap("y2", y2)

            # ---------------- FFN (GEGLU) ----------------
            ffT_pool = pb.enter_context(tc.tile_pool(name="ffT", bufs=1))
            ffT = [ffT_pool.tile([128, NC], BF16, name=f"ffT{i}", tag=f"ffT{i}")
                   for i in range(FF // 128)]
            with ExitStack() as pf:
                h3T_pool = pf.enter_context(tc.tile_pool(name="h3T", bufs=1))
                h3T = [h3T_pool.tile([128, NC], BF16, name=f"h3T{i}",
                                     tag=f"h3T{i}") for i in range(4)]
                psum_tr3 = pf.enter_context(
                    tc.tile_pool(name="ptr3", bufs=2, space="PSUM"))
                stat3 = pf.enter_context(tc.tile_pool(name="stat3", bufs=4))
                h3_pool = pf.enter_context(tc.tile_pool(name="hp3", bufs=6))
                _lnt(tc, pf, nc, y2, h3T, ident, psum_tr3, stat3, h3_pool,
                     name="l3")

                psum_g = pf.enter_context(
                    tc.tile_pool(name="pg", bufs=4, space="PSUM"))
                gl_pool = pf.enter_context(tc.tile_pool(name="gl", bufs=3))
                for gp in range(FF // 128):
                    for ic in range(2):
                        psv = psum_g.tile([128, 512], F32, name="psv", tag="pg")
                        psg = psum_g.tile([128, 512], F32, name="psg", tag="pg")
                        for cc in range(4):
                            nc.tensor.matmul(
                                psv[:],
                                lhsT=wfi_t[cc][:, gp * 128:(gp + 1) * 128],
                                rhs=h3T[cc][:, ic * 512:(ic + 1) * 512],
                                start=(cc == 0), stop=(cc == 3))
                        for cc in range(4):
                            nc.tensor.matmul(
                                psg[:],
                                lhsT=wfi_t[cc][:, FF + gp * 128:
                                               FF + (gp + 1) * 128],
                                rhs=h3T[cc][:, ic * 512:(ic + 1) * 512],
                                start=(cc == 0), stop=(cc == 3))
                        gl = gl_pool.tile([128, 512], BF16, name="glt",
                                          tag="gl")
                        nc.scalar.activation(gl[:], psg[:], AF.Gelu)
                        nc.vector.tensor_mul(
                            ffT[gp][:, ic * 512:(ic + 1) * 512], psv[:], gl[:])

            tap("ffT", ffT)

            # --- ff_out + residual -> DMA out ---
            with ExitStack() as pfo:
                psF = pfo.enter_context(
                    tc.tile_pool(name="psF", bufs=2, space="PSUM"))
                y3_pool = pfo.enter_context(tc.tile_pool(name="y3", bufs=3))
                for it in range(NCT):
                    ps = psF.tile([128, 512], F32, name="pf", tag="pf")
                    for gp in range(FF // 128):
                        nc.tensor.matmul(
                            ps[:],
                            lhsT=ffT[gp][:, it * 128:(it + 1) * 128],
                            rhs=wfo_t[gp][:],
                            start=(gp == 0), stop=(gp == FF // 128 - 1))
                    y3 = y3_pool.tile([128, D], F32, name="y3t", tag="y3")
                    nc.vector.tensor_add(y3[:], ps[:], y2[it][:])
                    nc.sync.dma_start(y[it * 128:(it + 1) * 128, :], y3[:])

    nc.compile()
    return nc


_CACHE = {}


def get_nc(debug_taps=False):
    key = ("nc", debug_taps)
    if key not in _CACHE:
        _CACHE[key] = build_nc(debug_taps)
    return _CACHE[key]


def make_in_maps(x, context, q1_w, k1_w, v1_w, o1_w, o1_b,
                 q2_w, k2_w, v2_w, o2_w, o2_b,
                 ff_in_w, ff_in_b, ff_out_w, ff_out_b,
                 ln1_g, ln1_b, ln2_g, ln2_b, ln3_g, ln3_b):
    for b_ in (o1_b, o2_b, ff_in_b, ff_out_b, ln1_b, ln2_b, ln3_b):
        assert not np.any(np.asarray(b_)), "nonzero biases not supported"
    bf = ml_dtypes.bfloat16
    wq1 = (np.asarray(ln1_g)[:, None] * np.asarray(q1_w) * SCALE).astype(bf)
    wk1 = (np.asarray(ln1_g)[:, None] * np.asarray(k1_w)).astype(bf)
    wv1 = (np.asarray(ln1_g)[:, None] * np.asarray(v1_w)).astype(bf)
    wo1 = np.asarray(o1_w).astype(bf)
    wq2 = (np.asarray(ln2_g)[:, None] * np.asarray(q2_w) * SCALE).astype(bf)
    wk2 = np.asarray(k2_w).astype(bf)
    wv2 = np.asarray(v2_w).astype(bf)
    wo2 = np.asarray(o2_w).astype(bf)
    wfi = (np.asarray(ln3_g)[:, None] * np.asarray(ff_in_w)).astype(bf)
    wfo = np.asarray(ff_out_w).astype(bf)
    ident = np.eye(128, dtype=bf)
    x = np.asarray(x, dtype=np.float32)
    ctxT = np.ascontiguousarray(
        np.asarray(context, dtype=np.float32).transpose(0, 2, 1)).astype(bf)

    in_maps = []
    for c in range(8):
        b_, ch = c // 4, c % 4
        in_maps.append({
            "x_full": np.ascontiguousarray(x[b_]),
            "x_own": np.ascontiguousarray(x[b_, ch * NC:(ch + 1) * NC]),
            "ctxT": np.ascontiguousarray(ctxT[b_]),
            "wq1": wq1, "wk1": wk1, "wv1": wv1, "wo1": wo1,
            "wq2": wq2, "wk2": wk2, "wv2": wv2, "wo2": wo2,
            "wfi": wfi, "wfo": wfo, "ident": ident,
        })
    return in_maps


def kernel(**inputs):
    nc = get_nc()
    in_maps = make_in_maps(**inputs)
    res = bass_utils.run_bass_kernel_spmd(nc, in_maps, core_ids=list(range(8)))
    out = np.empty((B, N, D), dtype=np.float32)
    for c in range(8):
        b_, ch = c // 4, c % 4
        out[b_, ch * NC:(ch + 1) * NC] = res.results[c]["y"]
    return out


# revision 41
# speedup vs baseline: 1.0096x; 1.0096x over previous
"""Trainium2 Bass kernel for a BasicTransformerBlock (self-attn + cross-attn +
GEGLU FFN), sharded over 8 NeuronCores.

Sharding: data-parallel over batch (2) x sequence chunks (4): core c handles
batch c//4, query rows [(c%4)*1024, (c%4+1)*1024). Each core recomputes
LN1+K/V over its batch's full 4096-row sequence (needed for self-attention)
and produces its own 1024-row output chunk. No collectives.

Layout convention on device: residual stream is token-major f32 [128_tok, 512]
tiles; matmul operands are bf16; activations are transposed to feature-major
[feat_part, tok_free] with PE transposes so the tensor engine can contract
over features; attention probabilities stay feature-major [key_part, query]
so exp'd tiles feed attn@v directly as the stationary operand.
"""

import numpy as np
import ml_dtypes
from contextlib import ExitStack

import concourse.bass as bass
import concourse.tile as tile
from concourse import bacc, mybir
from concourse import bass_utils

F32 = mybir.dt.float32
BF16 = mybir.dt.bfloat16
AF = mybir.ActivationFunctionType
ALU = mybir.AluOpType

# problem constants (hardcoded per the harness contract)
B = 2
N = 4096          # self-attn sequence length (per batch)
NC = 1024         # per-core query chunk
D = 512           # model dim
H = 8             # heads
DH = 64           # head dim
M = 256           # context length
CD = 768          # context dim
FF = 2048         # GEGLU inner dim
LN_EPS = 1e-5
SCALE = DH ** -0.5

NT = N // 128      # 32 x_full tiles
NCT = NC // 128    # 8 own tiles


def _lnt(tc, ctx, nc, src_tiles, hT, ident, psum_tr, stat_pool, h_pool,
         name=""):
    """LayerNorm (token-major f32 src tiles [128,512]) -> bf16, then PE
    transpose into feature-major hT tiles: hT[cc][:, t*128:(t+1)*128].
    src_tiles: list of SBUF tiles [128, 512] f32."""
    nt = len(src_tiles)
    hs = []
    for t, xt in enumerate(src_tiles):
        stats = stat_pool.tile([128, 6], F32, name=f"st{name}", tag="st")
        nc.vector.bn_stats(stats[:], xt[:])
        aggr = stat_pool.tile([128, 2], F32, name=f"ag{name}", tag="ag")
        nc.vector.bn_aggr(aggr[:], stats[:])
        veps = stat_pool.tile([128, 1], F32, name=f"ve{name}", tag="ve")
        nc.vector.tensor_scalar_add(veps[:], aggr[:, 1:2], LN_EPS)
        rvar = stat_pool.tile([128, 1], F32, name=f"rv{name}", tag="rv")
        nc.vector.reciprocal(rvar[:], veps[:])
        rstd = stat_pool.tile([128, 1], F32, name=f"rs{name}", tag="rs")
        nc.scalar.sqrt(rstd[:], rvar[:])
        h = h_pool.tile([128, D], BF16, name=f"h{name}", tag="h")
        nc.vector.tensor_scalar(h[:], xt[:], aggr[:, 0:1], rstd[:],
                                op0=ALU.subtract, op1=ALU.mult)
        hs.append(h)
    # transpose groups of 4 token-tiles at a time
    for tg in range((nt + 3) // 4):
        grp = list(range(tg * 4, min(tg * 4 + 4, nt)))
        for cc in range(4):
            ps = psum_tr.tile([128, 512], BF16, name=f"pst{name}", tag="pst")
            for k, t in enumerate(grp):
                nc.tensor.transpose(ps[:, k * 128:(k + 1) * 128],
                                    hs[t][:, cc * 128:(cc + 1) * 128],
                                    ident[:])
            w = len(grp) * 128
            nc.scalar.copy(hT[cc][:, tg * 512:tg * 512 + w], ps[:, :w])


def build_nc(debug_taps=False):
    nc = bacc.Bacc("TRN2", target_bir_lowering=False, debug=False,
                   enable_asserts=False, num_devices=8)
    dbg = {}

    def tap(name, tiles, rows=128):
        """DMA a list of SBUF tiles out to a debug DRAM tensor."""
        if not debug_taps:
            return
        ap0 = tiles[0][:]
        cols = ap0.shape[-1]
        dt = ap0.dtype
        t = nc.dram_tensor(f"dbg_{name}", [len(tiles) * rows, cols], dt,
                           kind="ExternalOutput").ap()
        for i, ti in enumerate(tiles):
            nc.sync.dma_start(t[i * rows:(i + 1) * rows, :], ti[:rows, :])
        dbg[name] = t
    dt_in = {}

    def din(name, shape, dt):
        dt_in[name] = nc.dram_tensor(name, shape, dt, kind="ExternalInput").ap()
        return dt_in[name]

    x_full = din("x_full", [N, D], F32)
    x_own = din("x_own", [NC, D], F32)
    ctxT = din("ctxT", [CD, M], BF16)
    wq1 = din("wq1", [D, D], BF16)
    wk1 = din("wk1", [D, D], BF16)
    wv1 = din("wv1", [D, D], BF16)
    wo1 = din("wo1", [D, D], BF16)
    wq2 = din("wq2", [D, D], BF16)
    wk2 = din("wk2", [CD, D], BF16)
    wv2 = din("wv2", [CD, D], BF16)
    wo2 = din("wo2", [D, D], BF16)
    wfi = din("wfi", [D, 2 * FF], BF16)
    wfo = din("wfo", [FF, D], BF16)
    ident_d = din("ident", [128, 128], BF16)
    y = nc.dram_tensor("y", [NC, D], F32, kind="ExternalOutput").ap()

    xf_t = x_full.rearrange("(t p) d -> t p d", p=128)
    xo_t = x_own.rearrange("(t p) d -> t p d", p=128)

    with tile.TileContext(nc) as tc, ExitStack() as top:
        const = top.enter_context(tc.tile_pool(name="const", bufs=1))
        ident = const.tile([128, 128], BF16)
        nc.sync.dma_start(ident[:], ident_d[:])

        y1_pool = top.enter_context(tc.tile_pool(name="y1", bufs=1))
        y1 = [y1_pool.tile([128, D], F32, name=f"y1_{i}", tag=f"y1_{i}")
              for i in range(NCT)]
        wq2p = top.enter_context(tc.tile_pool(name="wq2p", bufs=1))
        wq2_t = [wq2p.tile([128, D], BF16, name=f"wq2_{i}", tag=f"wq2_{i}")
                 for i in range(4)]
        for i in range(4):
            nc.sync.dma_start(wq2_t[i][:], wq2[i * 128:(i + 1) * 128, :])
        h2T_pool = top.enter_context(tc.tile_pool(name="h2T", bufs=1))
        h2T = [h2T_pool.tile([128, NC], BF16, name=f"h2T{i}", tag=f"h2T{i}")
               for i in range(4)]
        q2T_pool = top.enter_context(tc.tile_pool(name="q2T", bufs=1))
        q2T = [q2T_pool.tile([128, NC], BF16, name=f"q2T{i}", tag=f"q2T{i}")
               for i in range(4)]

        # ---------------- Phase A: self-attention ----------------
        with ExitStack() as pa:
            w1 = pa.enter_context(tc.tile_pool(name="w1", bufs=1))
            wq1_t = [w1.tile([128, D], BF16, name=f"wq1_{i}", tag=f"wq1_{i}") for i in range(4)]
            wk1_t = [w1.tile([128, D], BF16, name=f"wk1_{i}", tag=f"wk1_{i}") for i in range(4)]
            wv1_t = [w1.tile([128, D], BF16, name=f"wv1_{i}", tag=f"wv1_{i}") for i in range(4)]
            wo1_t = [w1.tile([128, D], BF16, name=f"wo1_{i}", tag=f"wo1_{i}") for i in range(4)]
            kT_pool = pa.enter_context(tc.tile_pool(name="kT", bufs=1))
            kT = [kT_pool.tile([128, N], BF16, name=f"kT{i}", tag=f"kT{i}")
                  for i in range(4)]
            va_pool = pa.enter_context(tc.tile_pool(name="va", bufs=1))
            va = [va_pool.tile([128, H * (DH + 1)], BF16, name=f"va{i}",
                               tag=f"va{i}") for i in range(NT)]
            qT_pool = pa.enter_context(tc.tile_pool(name="qT", bufs=1))
            qT = [qT_pool.tile([128, NC], BF16, name=f"qT{i}", tag=f"qT{i}")
                  for i in range(4)]
            xo_pool = pa.enter_context(tc.tile_pool(name="xo", bufs=1))
            xo = [xo_pool.tile([128, D], F32, name=f"xo{i}", tag=f"xo{i}")
                  for i in range(NCT)]
            for i in range(NCT):
                nc.sync.dma_start(xo[i][:], xo_t[i])
            oT_pool = pa.enter_context(tc.tile_pool(name="oT", bufs=1))
            oT = [[oT_pool.tile([128, 512], BF16, name=f"oT{d}_{i}",
                                tag=f"oT{d}_{i}") for i in range(2)]
                  for d in range(4)]

            # --- projections (scoped so h1T + psum free before attention) ---
            with ExitStack() as pp:
                h1T_pool = pp.enter_context(tc.tile_pool(name="h1T", bufs=1))
                h1T = [h1T_pool.tile([128, N], BF16, name=f"h1T{i}",
                                     tag=f"h1T{i}") for i in range(4)]
                hoT_pool = pp.enter_context(tc.tile_pool(name="hoT", bufs=1))
                hoT = [hoT_pool.tile([128, NC], BF16, name=f"hoT{i}",
                                     tag=f"hoT{i}") for i in range(4)]
                psum_tr = pp.enter_context(
                    tc.tile_pool(name="ptr", bufs=2, space="PSUM"))
                stat_pool = pp.enter_context(tc.tile_pool(name="stat", bufs=4))
                h_pool = pp.enter_context(tc.tile_pool(name="hp", bufs=6))
                xf_pool = pp.enter_context(tc.tile_pool(name="xf", bufs=4))

                xf_tiles = []
                for t in range(NT):
                    xt = xf_pool.tile([128, D], F32, name=f"xf{t}", tag="xf")
                    nc.sync.dma_start(xt[:], xf_t[t])
                    xf_tiles.append(xt)
                # weight DMAs issued after x so they don't delay the LN path
                for i in range(4):
                    nc.sync.dma_start(wq1_t[i][:], wq1[i * 128:(i + 1) * 128, :])
                    nc.sync.dma_start(wk1_t[i][:], wk1[i * 128:(i + 1) * 128, :])
                    nc.sync.dma_start(wv1_t[i][:], wv1[i * 128:(i + 1) * 128, :])
                    nc.sync.dma_start(wo1_t[i][:], wo1[i * 128:(i + 1) * 128, :])
                psum_pj = pp.enter_context(
                    tc.tile_pool(name="ppj", bufs=4, space="PSUM"))
                # fused LN1 + transpose + K/V projection per 512-token group:
                # keeps the PE fed with projection matmuls while the DVE works
                # on the next group's layernorm
                for tg in range(8):
                    grp = xf_tiles[tg * 4:(tg + 1) * 4]
                    hs = []
                    for xt in grp:
                        stats = stat_pool.tile([128, 6], F32, name="st1",
                                               tag="st")
                        nc.vector.bn_stats(stats[:], xt[:])
                        aggr = stat_pool.tile([128, 2], F32, name="ag1",
                                              tag="ag")
                        nc.vector.bn_aggr(aggr[:], stats[:])
                        veps = stat_pool.tile([128, 1], F32, name="ve1",
                                              tag="ve")
                        nc.vector.tensor_scalar_add(veps[:], aggr[:, 1:2],
                                                    LN_EPS)
                        rvar = stat_pool.tile([128, 1], F32, name="rv1",
                                              tag="rv")
                        nc.vector.reciprocal(rvar[:], veps[:])
                        rstd = stat_pool.tile([128, 1], F32, name="rs1",
                                              tag="rs")
                        nc.scalar.sqrt(rstd[:], rvar[:])
                        h = h_pool.tile([128, D], BF16, name="h1", tag="h")
                        nc.vector.tensor_scalar(h[:], xt[:], aggr[:, 0:1],
                                                rstd[:], op0=ALU.subtract,
                                                op1=ALU.mult)
                        hs.append(h)
                    for cc in range(4):
                        ps = psum_tr.tile([128, 512], BF16, name="pstr",
                                          tag="pst")
                        for k in range(4):
                            nc.tensor.transpose(
                                ps[:, k * 128:(k + 1) * 128],
                                hs[k][:, cc * 128:(cc + 1) * 128], ident[:])
                        nc.scalar.copy(
                            h1T[cc][:, tg * 512:(tg + 1) * 512], ps[:])
                    for dc in range(4):
                        ps = psum_pj.tile([128, 512], F32, name="pk", tag="pj")
                        for cc in range(4):
                            nc.tensor.matmul(
                                ps[:],
                                lhsT=wk1_t[cc][:, dc * 128:(dc + 1) * 128],
                                rhs=h1T[cc][:, tg * 512:(tg + 1) * 512],
                                start=(cc == 0), stop=(cc == 3))
                        nc.scalar.copy(
                            kT[dc][:, tg * 512:(tg + 1) * 512], ps[:])
                    for jt in range(tg * 4, (tg + 1) * 4):
                        ps = psum_pj.tile([128, 512], F32, name="pv", tag="pj")
                        for cc in range(4):
                            nc.tensor.matmul(
                                ps[:],
                                lhsT=h1T[cc][:, jt * 128:(jt + 1) * 128],
                                rhs=wv1_t[cc][:],
                                start=(cc == 0), stop=(cc == 3))
                        va_r = va[jt].rearrange("p (h e) -> p h e", e=DH + 1)
                        nc.vector.tensor_copy(
                            va_r[:, :, 0:DH],
                            ps[:].rearrange("p (h e) -> p h e", e=DH))
                        nc.gpsimd.memset(va_r[:, :, DH:DH + 1], 1.0)
                _lnt(tc, pp, nc, xo, hoT, ident, psum_tr, stat_pool,
                     h_pool, name="lo")
                # q1T
                for jg in range(NC // 512):
                    for dc in range(4):
                        ps = psum_pj.tile([128, 512], F32, name="pq", tag="pj")
                        for cc in range(4):
                            nc.tensor.matmul(
                                ps[:],
                                lhsT=wq1_t[cc][:, dc * 128:(dc + 1) * 128],
                                rhs=hoT[cc][:, jg * 512:(jg + 1) * 512],
                                start=(cc == 0), stop=(cc == 3))
                        nc.scalar.copy(
                            qT[dc][:, jg * 512:(jg + 1) * 512], ps[:])
                tap("h1T", h1T)
                tap("hoT", hoT)
            # --- attention main loop: ic-major with paired heads.
            # PSUM pools are scoped per ic chunk so that finish_ic (to_out1,
            # LN2, h2T, q2T) gets banks and overlaps the next chunk's
            # attention via Tile's dependency tracking. ---
            o_pool = pa.enter_context(tc.tile_pool(name="o_t", bufs=1))
            o_t = [[o_pool.tile([128, 4 * DH], BF16, name=f"o{h}_{i}",
                                tag=f"o{h}_{i}") for i in range(2)]
                   for h in range(H)]
            sm_pool = pa.enter_context(tc.tile_pool(name="sm", bufs=4))
            st2 = pa.enter_context(tc.tile_pool(name="st2", bufs=4))
            h2_pool = pa.enter_context(tc.tile_pool(name="hp2", bufs=6))
            pP = pa.enter_context(tc.tile_pool(name="pP", bufs=9))
            groups = [list(range(g * 3, min(g * 3 + 3, NT)))
                      for g in range((NT + 2) // 3)]
            WIN = 3

            def attn1_ic(ic, psS, psA):
                for hp in range(4):
                    oacc = [psA.tile([65, 512], F32, name=f"oa{hh}",
                                     tag="oa") for hh in range(2)]
                    for w0 in range(0, len(groups), WIN):
                        win = groups[w0:w0 + WIN]
                        pw = []
                        for grp in win:
                            w = len(grp) * 512
                            for hh in range(2):
                                base = 64 * hh
                                ps = psS.tile([128, 1536], F32, name="sim",
                                              tag="sim")
                                for k, jc in enumerate(grp):
                                    nc.tensor.matmul(
                                        ps[:, k * 512:(k + 1) * 512],
                                        lhsT=kT[hp][base:base + 64,
                                                    jc * 128:(jc + 1) * 128],
                                        rhs=qT[hp][base:base + 64,
                                                   ic * 512:(ic + 1) * 512],
                                        start=True, stop=True)
                                p = pP.tile([128, 1536], BF16, name="p",
                                            tag="p")
                                nc.scalar.activation(p[:, :w], ps[:, :w],
                                                     AF.Exp)
                                pw.append((hh, grp, p))
                        for hh, grp, p in pw:
                            h = 2 * hp + hh
                            for k, jc in enumerate(grp):
                                nc.tensor.matmul(
                                    oacc[hh][:],
                                    lhsT=va[jc][:, h * 65:h * 65 + 65],
                                    rhs=p[:, k * 512:(k + 1) * 512],
                                    start=(jc == 0), stop=(jc == NT - 1),
                                    skip_group_check=True)
                    for hh in range(2):
                        h = 2 * hp + hh
                        oc = sm_pool.tile([65, 512], BF16, name="oc",
                                          tag="oc")
                        nc.vector.tensor_copy(oc[:], oacc[hh][:])
                        pst = psA.tile([128, 264], BF16, name="pstt",
                                       tag="oa")
                        for m in range(4):
                            nc.tensor.transpose(
                                pst[:, m * 66:m * 66 + 65],
                                oc[:, m * 128:(m + 1) * 128],
                                ident[0:65, 0:65])
                        recip = sm_pool.tile([128, 4], F32, name="rc",
                                             tag="rc")
                        nc.vector.reciprocal(
                            recip[:],
                            pst[:, 0:264].rearrange(
                                "p (k e) -> p k e", e=66)[:, :, 64:65])
                        for m in range(4):
                            nc.vector.tensor_scalar(
                                o_t[h][ic][:, m * 64:(m + 1) * 64],
                                pst[:, m * 66:m * 66 + 64],
                                recip[:, m:m + 1], None, op0=ALU.mult)

            def finish_ic(ic, pfin):
                psU = pfin.enter_context(
                    tc.tile_pool(name=f"psU{ic}", bufs=2, space="PSUM"))
                for h in range(H):
                    dc, base = h // 2, 64 * (h % 2)
                    ps = psU.tile([64, 512], BF16, name="psO", tag="u")
                    for m in range(4):
                        nc.tensor.transpose(
                            ps[:, m * 128:(m + 1) * 128],
                            o_t[h][ic][:, m * 64:(m + 1) * 64], ident[:])
                    nc.vector.tensor_copy(
                        oT[dc][ic][base:base + 64, :], ps[:])
                h2s = []
                for m in range(4):
                    it = ic * 4 + m
                    ps = psU.tile([128, 512], F32, name="pu", tag="u")
                    for dc in range(4):
                        nc.tensor.matmul(
                            ps[:],
                            lhsT=oT[dc][ic][:, m * 128:(m + 1) * 128],
                            rhs=wo1_t[dc][:],
                            start=(dc == 0), stop=(dc == 3))
                    nc.vector.tensor_add(y1[it][:], ps[:], xo[it][:])
                    stats = st2.tile([128, 6], F32, name="st2", tag="st")
                    nc.vector.bn_stats(stats[:], y1[it][:])
                    aggr = st2.tile([128, 2], F32, name="ag2", tag="ag")
                    nc.vector.bn_aggr(aggr[:], stats[:])
                    veps = st2.tile([128, 1], F32, name="ve2", tag="ve")
                    nc.vector.tensor_scalar_add(veps[:], aggr[:, 1:2],
                                                LN_EPS)
                    rvar = st2.tile([128, 1], F32, name="rv2", tag="rv")
                    nc.vector.reciprocal(rvar[:], veps[:])
                    rstd = st2.tile([128, 1], F32, name="rs2", tag="rs")
                    nc.scalar.sqrt(rstd[:], rvar[:])
                    h2 = h2_pool.tile([128, D], BF16, name="h2", tag="h")
                    nc.vector.tensor_scalar(h2[:], y1[it][:],
                                            aggr[:, 0:1], rstd[:],
                                            op0=ALU.subtract, op1=ALU.mult)
                    h2s.append(h2)
                for cc in range(4):
                    ps = psU.tile([128, 512], BF16, name="ph2", tag="u")
                    for k in range(4):
                        nc.tensor.transpose(
                            ps[:, k * 128:(k + 1) * 128],
                            h2s[k][:, cc * 128:(cc + 1) * 128], ident[:])
                    nc.scalar.copy(
                        h2T[cc][:, ic * 512:(ic + 1) * 512], ps[:])
                for dc in range(4):
                    ps = psU.tile([128, 512], F32, name="pq2", tag="u")
                    for cc in range(4):
                        nc.tensor.matmul(
                            ps[:],
                            lhsT=wq2_t[cc][:, dc * 128:(dc + 1) * 128],
                            rhs=h2T[cc][:, ic * 512:(ic + 1) * 512],
                            start=(cc == 0), stop=(cc == 3))
                    nc.scalar.copy(
                        q2T[dc][:, ic * 512:(ic + 1) * 512], ps[:])

            with ExitStack() as pat:
                psS = pat.enter_context(
                    tc.tile_pool(name="psS", bufs=2, space="PSUM"))
                psA = pat.enter_context(
                    tc.tile_pool(name="psA", bufs=2, space="PSUM"))
                attn1_ic(0, psS, psA)
                attn1_ic(1, psS, psA)
            with ExitStack() as pfin:
                finish_ic(0, pfin)
                finish_ic(1, pfin)

        tap("y1", y1)

        # ---------------- Phase B: cross-attention + FFN ----------------
        with ExitStack() as pb:
            wf = pb.enter_context(tc.tile_pool(name="wf", bufs=1))
            wfi_t = [wf.tile([128, 2 * FF], BF16, name=f"wfi{i}", tag=f"wfi{i}")
                     for i in range(4)]
            wfo_t = [wf.tile([128, D], BF16, name=f"wfo{i}", tag=f"wfo{i}")
                     for i in range(FF // 128)]
            for i in range(4):
                nc.sync.dma_start(wfi_t[i][:], wfi[i * 128:(i + 1) * 128, :])
            for i in range(FF // 128):
                nc.sync.dma_start(wfo_t[i][:], wfo[i * 128:(i + 1) * 128, :])

            w2 = pb.enter_context(tc.tile_pool(name="w2", bufs=1))
            wk2_t = [w2.tile([128, D], BF16, name=f"wk2_{i}", tag=f"wk2_{i}") for i in range(6)]
            wv2_t = [w2.tile([128, D], BF16, name=f"wv2_{i}", tag=f"wv2_{i}") for i in range(6)]
            wo2_t = [w2.tile([128, D], BF16, name=f"wo2_{i}", tag=f"wo2_{i}") for i in range(4)]
            ctx_t = [w2.tile([128, M], BF16, name=f"ctx{i}", tag=f"ctx{i}") for i in range(6)]
            for i in range(4):
                nc.sync.dma_start(wo2_t[i][:], wo2[i * 128:(i + 1) * 128, :])
            for i in range(6):
                nc.sync.dma_start(wk2_t[i][:], wk2[i * 128:(i + 1) * 128, :])
                nc.sync.dma_start(wv2_t[i][:], wv2[i * 128:(i + 1) * 128, :])
                nc.sync.dma_start(ctx_t[i][:], ctxT[i * 128:(i + 1) * 128, :])

            y2_pool = pb.enter_context(tc.tile_pool(name="y2", bufs=1))
            y2 = [y2_pool.tile([128, D], F32, name=f"y2_{i}", tag=f"y2_{i}")
                  for i in range(NCT)]

            k2T_pool = pb.enter_context(tc.tile_pool(name="k2T", bufs=1))
            k2T = [k2T_pool.tile([128, M], BF16, name=f"k2T{i}", tag=f"k2T{i}")
                   for i in range(4)]
            va2_pool = pb.enter_context(tc.tile_pool(name="va2", bufs=1))
            va2 = [va2_pool.tile([128, H * (DH + 1)], BF16, name=f"va2_{i}",
                                 tag=f"va2_{i}") for i in range(2)]
            o2T_pool = pb.enter_context(tc.tile_pool(name="o2T", bufs=1))
            o2T = [[o2T_pool.tile([128, 512], BF16, name=f"o2T{d}_{i}",
                                  tag=f"o2T{d}_{i}") for i in range(2)]
                   for d in range(4)]

            # --- projections for cross-attn (k2/v2 from context only;
            # h2T/q2T were produced during the attention overlap) ---
            with ExitStack() as pp2:
                psum_p2 = pp2.enter_context(
                    tc.tile_pool(name="pp2", bufs=4, space="PSUM"))
                for dc in range(4):
                    ps = psum_p2.tile([128, M], F32, name="pk2", tag="p2")
                    for cc in range(6):
                        nc.tensor.matmul(
                            ps[:],
                            lhsT=wk2_t[cc][:, dc * 128:(dc + 1) * 128],
                            rhs=ctx_t[cc][:],
                            start=(cc == 0), stop=(cc == 5))
                    nc.vector.tensor_copy(k2T[dc][:], ps[:])
                for jt in range(2):
                    ps = psum_p2.tile([128, 512], F32, name="pv2", tag="p2")
                    for cc in range(6):
                        nc.tensor.matmul(
                            ps[:],
                            lhsT=ctx_t[cc][:, jt * 128:(jt + 1) * 128],
                            rhs=wv2_t[cc][:],
                            start=(cc == 0), stop=(cc == 5))
                    va_r = va2[jt].rearrange("p (h e) -> p h e", e=DH + 1)
                    nc.vector.tensor_copy(
                        va_r[:, :, 0:DH],
                        ps[:].rearrange("p (h e) -> p h e", e=DH))
                    nc.gpsimd.memset(va_r[:, :, DH:DH + 1], 1.0)

            # --- cross-attention loop (2 key tiles) ---
            o2_pool = pb.enter_context(tc.tile_pool(name="o2_t", bufs=1))
            o2_t = [[o2_pool.tile([128, 4 * DH], BF16, name=f"o2{h}_{i}",
                                  tag=f"o2{h}_{i}") for i in range(2)]
                    for h in range(H)]
            with ExitStack() as pat2:
                psS2 = pat2.enter_context(
                    tc.tile_pool(name="psS2", bufs=2, space="PSUM"))
                psA2 = pat2.enter_context(
                    tc.tile_pool(name="psA2", bufs=2, space="PSUM"))
                pP2 = pat2.enter_context(tc.tile_pool(name="pP2", bufs=4))
                sm2 = pat2.enter_context(tc.tile_pool(name="sm2", bufs=4))
                for hp in range(4):
                    for ic in range(2):
                        oacc = [psA2.tile([65, 512], F32, name=f"o2a{hh}",
                                          tag="o2a") for hh in range(2)]
                        for hh in range(2):
                            h = 2 * hp + hh
                            base = 64 * hh
                            ps = psS2.tile([128, 1024], F32, name="sim2",
                                           tag="sim2")
                            for jc in range(2):
                                nc.tensor.matmul(
                                    ps[:, jc * 512:(jc + 1) * 512],
                                    lhsT=k2T[hp][base:base + 64,
                                                 jc * 128:(jc + 1) * 128],
                                    rhs=q2T[hp][base:base + 64,
                                                ic * 512:(ic + 1) * 512],
                                    start=True, stop=True)
                            p = pP2.tile([128, 1024], BF16, name="p2", tag="p2")
                            nc.scalar.activation(p[:], ps[:], AF.Exp)
                            for jc in range(2):
                                nc.tensor.matmul(
                                    oacc[hh][:],
                                    lhsT=va2[jc][:, h * 65:h * 65 + 65],
                                    rhs=p[:, jc * 512:(jc + 1) * 512],
                                    start=(jc == 0), stop=(jc == 1),
                                    skip_group_check=True)
                        for hh in range(2):
                            h = 2 * hp + hh
                            oc = sm2.tile([65, 512], BF16, name="oc2",
                                          tag="oc2")
                            nc.vector.tensor_copy(oc[:], oacc[hh][:])
                            pst = psA2.tile([128, 264], BF16, name="pstt2",
                                            tag="o2a")
                            for m in range(4):
                                nc.tensor.transpose(
                                    pst[:, m * 66:m * 66 + 65],
                                    oc[:, m * 128:(m + 1) * 128],
                                    ident[0:65, 0:65])
                            recip = sm2.tile([128, 4], F32, name="rc2",
                                             tag="rc2")
                            nc.vector.reciprocal(
                                recip[:],
                                pst[:, 0:264].rearrange(
                                    "p (k e) -> p k e", e=66)[:, :, 64:65])
                            for m in range(4):
                                nc.vector.tensor_scalar(
                                    o2_t[h][ic][:, m * 64:(m + 1) * 64],
                                    pst[:, m * 66:m * 66 + 64],
                                    recip[:, m:m + 1], None, op0=ALU.mult)

            with ExitStack() as pot2:
                psO2 = pot2.enter_context(
                    tc.tile_pool(name="psO2", bufs=4, space="PSUM"))
                for h in range(H):
                    dc, base = h // 2, 64 * (h % 2)
                    for ic in range(2):
                        ps = psO2.tile([64, 512], BF16, name="pso2", tag="pso2")
                        for m in range(4):
                            nc.tensor.transpose(
                                ps[:, m * 128:(m + 1) * 128],
                                o2_t[h][ic][:, m * 64:(m + 1) * 64],
                                ident[:])
                        nc.vector.tensor_copy(
                            o2T[dc][ic][base:base + 64, :], ps[:])

            with ExitStack() as pto2:
                psU2 = pto2.enter_context(
                    tc.tile_pool(name="psU2", bufs=2, space="PSUM"))
                for ic in range(2):
                    for m in range(4):
                        it = ic * 4 + m
                        ps = psU2.tile([128, 512], F32, name="pu2", tag="pu2")
                        for dc in range(4):
                            nc.tensor.matmul(
                                ps[:],
                                lhsT=o2T[dc][ic][:, m * 128:(m + 1) * 128],
                                rhs=wo2_t[dc][:],
                                start=(dc == 0), stop=(dc == 3))
                        nc.vector.tensor_add(y2[it][:], ps[:], y1[it][:])

            tap("k2T", k2T)
            tap("q2T", q2T)
            tap("va2", va2)
            tap("o2_t", [o2_t[h][i] for h in range(H) for i in range(2)])
            tap("y2", y2)

            # ---------------- FFN (GEGLU) ----------------
            ffT_pool = pb.enter_context(tc.tile_pool(name="ffT", bufs=1))
            ffT = [ffT_pool.tile([128, NC], BF16, name=f"ffT{i}", tag=f"ffT{i}")
                   for i in range(FF // 128)]
            with ExitStack() as pf:
                h3T_pool = pf.enter_context(tc.tile_pool(name="h3T", bufs=1))
                h3T = [h3T_pool.tile([128, NC], BF16, name=f"h3T{i}",
                                     tag=f"h3T{i}") for i in range(4)]
                psum_tr3 = pf.enter_context(
                    tc.tile_pool(name="ptr3", bufs=2, space="PSUM"))
                stat3 = pf.enter_context(tc.tile_pool(name="stat3", bufs=4))
                h3_pool = pf.enter_context(tc.tile_pool(name="hp3", bufs=6))
                _lnt(tc, pf, nc, y2, h3T, ident, psum_tr3, stat3, h3_pool,
                     name="l3")

                psum_g = pf.enter_context(
                    tc.tile_pool(name="pg", bufs=4, space="PSUM"))
                gl_pool = pf.enter_context(tc.tile_pool(name="gl", bufs=3))
                for gp in range(FF // 128):
                    for ic in range(2):
                        psv = psum_g.tile([128, 512], F32, name="psv", tag="pg")
                        psg = psum_g.tile([128, 512], F32, name="psg", tag="pg")
                        for cc in range(4):
                            nc.tensor.matmul(
                                psv[:],
                                lhsT=wfi_t[cc][:, gp * 128:(gp + 1) * 128],
                                rhs=h3T[cc][:, ic * 512:(ic + 1) * 512],
                                start=(cc == 0), stop=(cc == 3))
                        for cc in range(4):
                            nc.tensor.matmul(
                                psg[:],
                                lhsT=wfi_t[cc][:, FF + gp * 128:
                                               FF + (gp + 1) * 128],
                                rhs=h3T[cc][:, ic * 512:(ic + 1) * 512],
                                start=(cc == 0), stop=(cc == 3))
                        gl = gl_pool.tile([128, 512], BF16, name="glt",
                                          tag="gl")
                        nc.scalar.activation(gl[:], psg[:], AF.Gelu)
                        nc.vector.tensor_mul(
                            ffT[gp][:, ic * 512:(ic + 1) * 512], psv[:], gl[:])

            tap("ffT", ffT)

            # --- ff_out + residual -> DMA out ---
            with ExitStack() as pfo:
                psF = pfo.enter_context(
                    tc.tile_pool(name="psF", bufs=2, space="PSUM"))
                y3_pool = pfo.enter_context(tc.tile_pool(name="y3", bufs=3))
                for it in range(NCT):
                    ps = psF.tile([128, 512], F32, name="pf", tag="pf")
                    for gp in range(FF // 128):
                        nc.tensor.matmul(
                            ps[:],
                            lhsT=ffT[gp][:, it * 128:(it + 1) * 128],
                            rhs=wfo_t[gp][:],
                            start=(gp == 0), stop=(gp == FF // 128 - 1))
                    y3 = y3_pool.tile([128, D], F32, name="y3t", tag="y3")
                    nc.vector.tensor_add(y3[:], ps[:], y2[it][:])
                    nc.sync.dma_start(y[it * 128:(it + 1) * 128, :], y3[:])

    nc.compile()
    return nc


_CACHE = {}


def get_nc(debug_taps=False):
    key = ("nc", debug_taps)
    if key not in _CACHE:
        _CACHE[key] = build_nc(debug_taps)
    return _CACHE[key]


def make_in_maps(x, context, q1_w, k1_w, v1_w, o1_w, o1_b,
                 q2_w, k2_w, v2_w, o2_w, o2_b,
                 ff_in_w, ff_in_b, ff_out_w, ff_out_b,
                 ln1_g, ln1_b, ln2_g, ln2_b, ln3_g, ln3_b):
    for b_ in (o1_b, o2_b, ff_in_b, ff_out_b, ln1_b, ln2_b, ln3_b):
        assert not np.any(np.asarray(b_)), "nonzero biases not supported"
    bf = ml_dtypes.bfloat16
    wq1 = (np.asarray(ln1_g)[:, None] * np.asarray(q1_w) * SCALE).astype(bf)
    wk1 = (np.asarray(ln1_g)[:, None] * np.asarray(k1_w)).astype(bf)
    wv1 = (np.asarray(ln1_g)[:, None] * np.asarray(v1_w)).astype(bf)
    wo1 = np.asarray(o1_w).astype(bf)
    wq2 = (np.asarray(ln2_g)[:, None] * np.asarray(q2_w) * SCALE).astype(bf)
    wk2 = np.asarray(k2_w).astype(bf)
    wv2 = np.asarray(v2_w).astype(bf)
    wo2 = np.asarray(o2_w).astype(bf)
    wfi = (np.asarray(ln3_g)[:, None] * np.asarray(ff_in_w)).astype(bf)
    wfo = np.asarray(ff_out_w).astype(bf)
    ident = np.eye(128, dtype=bf)
    x = np.asarray(x, dtype=np.float32)
    ctxT = np.ascontiguousarray(
        np.asarray(context, dtype=np.float32).transpose(0, 2, 1)).astype(bf)

    in_maps = []
    for c in range(8):
        b_, ch = c // 4, c % 4
        in_maps.append({
            "x_full": np.ascontiguousarray(x[b_]),
            "x_own": np.ascontiguousarray(x[b_, ch * NC:(ch + 1) * NC]),
            "ctxT": np.ascontiguousarray(ctxT[b_]),
            "wq1": wq1, "wk1": wk1, "wv1": wv1, "wo1": wo1,
            "wq2": wq2, "wk2": wk2, "wv2": wv2, "wo2": wo2,
            "wfi": wfi, "wfo": wfo, "ident": ident,
        })
    return in_maps


def kernel(**inputs):
    nc = get_nc()
    in_maps = make_in_maps(**inputs)
    res = bass_utils.run_bass_kernel_spmd(nc, in_maps, core_ids=list(range(8)))
    out = np.empty((B, N, D), dtype=np.float32)
    for c in range(8):
        b_, ch = c // 4, c % 4
        out[b_, ch * NC:(ch + 1) * NC] = res.results[c]["y"]
    return out
